# revision 15
# baseline (speedup 1.0000x reference)
"""Causal self-attention Trainium2 kernel (Bass/Tile), 8 NeuronCores.

Problem: B=2, S=2048, D=1024, H=16 heads (hd=64), fp32.
    qkv = x @ qkv_w + qkv_b ; per-head causal attention ; y = out @ out_proj + out_b

Sharding (hybrid data x tensor parallel):
    8 cores = 2 batch groups x 4 head groups. Core c handles batch c//4 and
    the 4 heads [4*(c%4) .. 4*(c%4)+3]. Each core computes its partial
    out-projection y_c [S, D] in bf16; host sums the 4 partials per batch
    (in fp32) + out_b.

v2 design (bf16 everywhere on the PE, balanced engine usage):
    - all matmuls in bf16 (full PE rate at any free size, half the DMA/SBUF)
    - phase 1 (projections) streams x per-ko with ko-OUTER accumulation over
      8 PSUM banks so the PE starts as soon as x[0] lands; PSUM->SBUF drains
      run on the otherwise-idle ACT engine
    - softmax: exp on ACT only (no Ln -> no activation-table thrash);
      denominators via the ones-column of V; 1/l via DVE
      reciprocal_approx_fast (~5x faster than nc.vector.reciprocal);
      broadcast of 1/l across partitions via a tiny K=2 f32r matmul
    - causal masking of diagonal 128-blocks via gpsimd.affine_select on the
      exp'd tile (keeps DVE free)
    - softmax-normalize + out-projection of q-tile jq are emitted as deferred
      units interleaved into the next tile's attention loop, so the PE never
      stalls on the normalization chain
"""

import os
import sys
from collections import deque

for _p in ("/opt/trn_rl_repo", "/root/.axon_site/_ro/trn_rl_repo"):
    if os.path.isdir(_p) and _p not in sys.path:
        sys.path.insert(0, _p)

import numpy as np
import ml_dtypes
from contextlib import ExitStack

import concourse.bass as bass
import concourse.tile as tile
from concourse import bacc, mybir
from concourse.bass_utils import run_bass_kernel_spmd

B, S, D = 2, 2048, 1024
H, HD = 16, 64
NCORES = 8
LOCAL_H = 4           # heads per core
P = 128
KO = D // P           # 8 contraction sub-tiles for the projections
NQ = S // 512         # 4 q-tiles of 512
NKT = S // P          # 16 k-blocks of 128
F32 = mybir.dt.float32
F32R = mybir.dt.float32r
BF16 = mybir.dt.bfloat16
AF = mybir.ActivationFunctionType
ALU = mybir.AluOpType
SCALE = 1.0 / np.sqrt(HD)


def _emit(tc, nc, xT, wqk, wv, wo, bqk, b65, onesd, sel2, y, has_qkv_bias):
    with ExitStack() as ctx:
        consts = ctx.enter_context(tc.tile_pool(name="consts", bufs=1))
        persis = ctx.enter_context(tc.tile_pool(name="persist", bufs=1))
        xstack = ctx.enter_context(ExitStack())
        xpool = xstack.enter_context(tc.tile_pool(name="xp", bufs=KO))
        ppstack = ctx.enter_context(ExitStack())
        pp = ppstack.enter_context(tc.tile_pool(name="pp", bufs=8, space="PSUM"))

        # ---- loads: wqk + early x tiles first (weights for later phases
        # are issued after the x stream so they don't steal DMA bandwidth)
        wqk_sb = consts.tile([P, KO, 512], BF16)
        nc.gpsimd.dma_start(wqk_sb[:], wqk.rearrange("(ko p) m -> p ko m", p=P))
        x_sb = []
        for ko in range(KO):
            t = xpool.tile([P, S], BF16, tag="x", name=f"x{ko}")
            x_sb.append(t)
        for ko in range(4):
            nc.sync.dma_start(x_sb[ko][:], xT[ko * P:(ko + 1) * P, :])
        b65_sb = consts.tile([1, 260], BF16)
        nc.scalar.dma_start(b65_sb[:], b65[None, :])
        ones_sb = consts.tile([1, P], BF16)
        nc.scalar.dma_start(ones_sb[:], onesd[None, :])
        onesr_sb = consts.tile([1, P], F32R)
        nc.scalar.dma_start(onesr_sb[:], sel2[0:1, :])
        for ko in range(4, KO):
            nc.sync.dma_start(x_sb[ko][:], xT[ko * P:(ko + 1) * P, :])
        wv_sb = consts.tile([P, KO, 260], BF16)
        nc.scalar.dma_start(wv_sb[:], wv.rearrange("(ko p) m -> p ko m", p=P))
        wo_sb = consts.tile([P, 2, D], BF16)
        nc.scalar.dma_start(wo_sb[:], wo.rearrange("(ks p) n -> p ks n", p=P))
        if has_qkv_bias:
            bqk_sb = consts.tile([P, 4], F32)
            nc.scalar.dma_start(bqk_sb[:], bqk.rearrange("(m p) -> p m", p=P))

        # persistent activations
        qkT = persis.tile([P, 4, S], BF16)       # m 0,1: qT(h0..h3); 2,3: kT
        v_all = persis.tile([P, NKT, LOCAL_H, 65], BF16)  # [k-part, kt, lh, hd|ones]
        outT = persis.tile([P, 2, S], BF16)      # attention out^T (out-proj lhsT)

        # ---- phase 1: projections, ko-outer over 8 PSUM banks ----
        # qkT[m] = (wqk[:, m-slice]).T @ xT, two passes of 8 (m, n) groups.
        # Each pass is ko-split (0-3, then 4-7) so the PE starts as soon as
        # the first x tiles land instead of waiting for the full x stream.
        for half in range(2):
            groups = [(m, 2 * half + nn) for m in range(4) for nn in range(2)]
            ts = [pp.tile([P, 512], F32, tag="p", name=f"qk{half}_{g}")
                  for g in range(8)]
            for ko in range(KO):
                for g, (m, n) in enumerate(groups):
                    nc.tensor.matmul(
                        ts[g][:],
                        wqk_sb[:, ko, m * P:(m + 1) * P],
                        x_sb[ko][:, n * 512:(n + 1) * 512],
                        start=(ko == 0), stop=(ko == KO - 1),
                    )
            for g, (m, n) in enumerate(groups):
                dst = qkT[:, m, n * 512:(n + 1) * 512]
                if has_qkv_bias:
                    nc.scalar.activation(dst, ts[g][:], AF.Identity,
                                         bias=bqk_sb[:, m:m + 1])
                else:
                    nc.scalar.copy(dst, ts[g][:])

        # v projection (natural layout, ones/bias row via K=1 matmul).
        # Second half drains on DVE so the ACT queue is clear for the first
        # attention exp.
        for half in range(2):
            mts = [8 * half + g for g in range(8)]
            ts = [pp.tile([P, 512], F32, tag="p", name=f"v{half}_{g}")
                  for g in range(8)]
            for ko in range(KO):
                for g, mt in enumerate(mts):
                    nc.tensor.matmul(
                        ts[g][:, 0:260],
                        x_sb[ko][:, mt * P:(mt + 1) * P],
                        wv_sb[:, ko, :],
                        start=(ko == 0), stop=False,
                    )
            for g, mt in enumerate(mts):
                nc.tensor.matmul(ts[g][:, 0:260], ones_sb[:1, :], b65_sb[:1, :],
                                 start=False, stop=True)
                (nc.scalar.copy if half == 0 else nc.vector.tensor_copy)(
                    v_all[:, mt, :, :],
                    ts[g][:, 0:260].rearrange("p (h d) -> p h d", h=LOCAL_H),
                )

        # x tiles + phase-1 psum are dead; release for the attention pools
        xstack.close()
        ppstack.close()

        psA = ctx.enter_context(tc.tile_pool(name="psA", bufs=3, space="PSUM"))
        psB = ctx.enter_context(tc.tile_pool(name="psB", bufs=2, space="PSUM"))
        work = ctx.enter_context(tc.tile_pool(name="work", bufs=4))
        small = ctx.enter_context(tc.tile_pool(name="small", bufs=2))
        ypool = ctx.enter_context(tc.tile_pool(name="yp", bufs=4))

        units = deque()

        def unit_rb_norm(jq, hp, st, rrr, i):
            def emit():
                rbp = psA.tile([P, 2, 512], F32, tag="s", name="rbp")
                nc.tensor.matmul(rbp[0:64, 0, :], onesr_sb[:1, 0:64],
                                 rrr[:1, :], start=True, stop=True)
                nc.vector.tensor_tensor(
                    outT[64 * i:64 * i + 64, hp, jq * 512:(jq + 1) * 512],
                    st[0:64, :], rbp[0:64, 0, :], ALU.mult)
            return emit

        def unit_outproj(jq, mt, n2):
            def emit():
                pso = psA.tile([P, 2, 512], F32, tag="s", name="pso")[:, 0, :]
                for ks in range(2):
                    nc.tensor.matmul(
                        pso,
                        outT[:, ks, mt * P:(mt + 1) * P],
                        wo_sb[:, ks, n2 * 512:(n2 + 1) * 512],
                        start=(ks == 0), stop=(ks == 1),
                    )
                yt = ypool.tile([P, 512], BF16, tag="y", name="yt")
                nc.vector.tensor_copy(yt[:], pso)
                nc.sync.dma_start(
                    y[mt * P:(mt + 1) * P, n2 * 512:(n2 + 1) * 512], yt[:])
            return emit

        def emit_pv(po, jq, hp, kt, et, f0, last_kt):
            for i in range(2):
                nc.tensor.matmul(
                    po[i][:, f0:512],
                    v_all[:, kt, 2 * hp + i, :],
                    et[:, i, f0:512],
                    start=(kt == 0), stop=(kt == last_kt),
                )

        # ---- phase 2: attention with deferred normalize/out-proj units ----
        for jq in range(NQ):
            for hp in range(2):
                last_kt = 4 * jq + 3
                po = [psB.tile([65, 512], F32, tag="o", name=f"po{jq}{hp}{i_}")
                      for i_ in range(2)]
                pend = deque()
                for kt in range(last_kt + 1):
                    rel = kt - 4 * jq
                    f0 = 128 * rel if rel > 0 else 0
                    ps = psA.tile([P, 2, 512], F32, tag="s", name="ps")
                    for i in range(2):
                        poff = 64 * i
                        nc.tensor.matmul(
                            ps[:, i, f0:512],
                            qkT[poff:poff + 64, 2 + hp, kt * P:(kt + 1) * P],
                            qkT[poff:poff + 64, hp,
                                jq * 512 + f0:(jq + 1) * 512],
                            start=True, stop=True,
                        )
                    et = work.tile([P, 2, 512], BF16, tag="e", name="et")
                    nc.scalar.activation(et[:, :, f0:512], ps[:, :, f0:512],
                                         AF.Exp, scale=float(SCALE))
                    if rel >= 0:   # zero the 128-wide triangle at [f0, f0+128)
                        nc.gpsimd.affine_select(
                            out=et[:, :, f0:f0 + 128],
                            in_=et[:, :, f0:f0 + 128],
                            pattern=[[0, 2], [1, P]],
                            compare_op=ALU.is_ge, fill=0.0, base=0,
                            channel_multiplier=-1,
                        )
                    pend.append((kt, et, f0))
                    if len(pend) > 2:   # 2-deep score lookahead ahead of PV
                        emit_pv(po, jq, hp, *pend.popleft(), last_kt)
                    if units:   # fill PE with deferred work every iteration
                        units.popleft()()
                while pend:
                    emit_pv(po, jq, hp, *pend.popleft(), last_kt)
                # stage po out of PSUM on the ACT engine (frees the bank
                # fast), grab denominators, 1/l via fast DVE approx
                for i in range(2):
                    stt = work.tile([65, 512], F32, tag="st", name="st")
                    nc.vector.tensor_copy(stt[:], po[i][:])
                    lcp = small.tile([1, 512], F32, tag="lcp", name="lcp")
                    nc.vector.tensor_copy(lcp[:], po[i][64:65, :])
                    rr = small.tile([1, 512], F32, tag="rr", name="rr")
                    nc.vector.reciprocal_approx_fast(rr[:], lcp[:])
                    rrr = small.tile([1, 512], F32R, tag="rrr", name="rrr")
                    nc.vector.tensor_copy(rrr[:], rr[:])
                    units.append(unit_rb_norm(jq, hp, stt, rrr, i))
            for mt in range(4 * jq, 4 * jq + 4):
                for n2 in range(2):
                    units.append(unit_outproj(jq, mt, n2))
        while units:
            units.popleft()()


def build_nc(has_qkv_bias):
    nc = bacc.Bacc("TRN2", target_bir_lowering=False, debug=False,
                   num_devices=NCORES)
    xT = nc.dram_tensor("xT", [D, S], BF16, kind="ExternalInput")
    wqk = nc.dram_tensor("wqk", [D, 512], BF16, kind="ExternalInput")
    wv = nc.dram_tensor("wv", [D, 260], BF16, kind="ExternalInput")
    wo = nc.dram_tensor("wo", [2 * P, D], BF16, kind="ExternalInput")
    bqk = nc.dram_tensor("bqk", [512], F32, kind="ExternalInput")
    b65 = nc.dram_tensor("b65", [260], BF16, kind="ExternalInput")
    onesd = nc.dram_tensor("onesd", [P], BF16, kind="ExternalInput")
    sel2 = nc.dram_tensor("sel2", [2, P], F32R, kind="ExternalInput")
    y = nc.dram_tensor("y", [S, D], BF16, kind="ExternalOutput")
    with tile.TileContext(nc) as tc:
        _emit(tc, nc, xT.ap(), wqk.ap(), wv.ap(), wo.ap(), bqk.ap(), b65.ap(),
              onesd.ap(), sel2.ap(), y.ap(), has_qkv_bias)
    nc.compile()
    return nc


_NC_CACHE = {}


def _get_nc(has_qkv_bias):
    key = bool(has_qkv_bias)
    if key not in _NC_CACHE:
        _NC_CACHE[key] = build_nc(key)
    return _NC_CACHE[key]


def _bf16(a):
    return np.ascontiguousarray(a, dtype=np.float32).astype(ml_dtypes.bfloat16)


def make_in_maps(x, qkv_w, qkv_b, out_w):
    """Per-core host-side sharding. Core c: batch c//4, heads 4*(c%4)..+3."""
    in_maps = []
    xTs = [_bf16(x[b].T) for b in range(B)]
    sel2 = np.zeros((2, P), np.float32)
    sel2[0, 0:64] = 1.0
    sel2[1, 64:128] = 1.0
    for c in range(NCORES):
        b = c // (NCORES // B)
        g = c % (NCORES // B)
        h0 = LOCAL_H * g
        cols = slice(h0 * HD, (h0 + LOCAL_H) * HD)
        wq = qkv_w[:, cols]
        wk = qkv_w[:, D:][:, cols]
        wv_ = qkv_w[:, 2 * D:][:, cols]
        bq = qkv_b[cols]
        bk = qkv_b[D:][cols]
        bv = qkv_b[2 * D:][cols]
        wv_pad = np.zeros((D, LOCAL_H, 65), np.float32)
        wv_pad[:, :, :64] = wv_.reshape(D, LOCAL_H, HD)
        b65_arr = np.zeros((LOCAL_H, 65), np.float32)
        b65_arr[:, :64] = bv.reshape(LOCAL_H, HD)
        b65_arr[:, 64] = 1.0
        in_maps.append({
            "xT": xTs[b],
            "wqk": _bf16(np.concatenate([wq, wk], axis=1)),
            "wv": _bf16(wv_pad.reshape(D, LOCAL_H * 65)),
            "wo": _bf16(out_w[cols, :]),
            "bqk": np.ascontiguousarray(np.concatenate([bq, bk])),
            "b65": _bf16(b65_arr.reshape(-1)),
            "onesd": np.ones(P, ml_dtypes.bfloat16),
            "sel2": sel2,
        })
    return in_maps


def _ensure_ntff_hook():
    """Provide antenv.axon_hooks (missing in this image) so trace=True works."""
    try:
        from antenv.axon_hooks import get_axon_ntff_profile_hook  # noqa: F401
        return
    except ImportError:
        pass
    import types
    import antenv
    mod = types.ModuleType("antenv.axon_hooks")
    holder = {"hook": None}
    mod.set_axon_ntff_profile_hook = lambda h: holder.__setitem__("hook", h)
    mod.get_axon_ntff_profile_hook = lambda: holder["hook"]
    sys.modules["antenv.axon_hooks"] = mod
    antenv.axon_hooks = mod
    try:
        from trn_agent_boot.trn_boot import _ntff_profile_via_ctypes
        so = "/opt/axon/libaxon_pjrt.so"
        if os.path.exists(so):
            mod.set_axon_ntff_profile_hook(_ntff_profile_via_ctypes(so))
    except Exception:
        pass


def kernel(x, qkv_w, qkv_b, out_w, out_b, _trace=False):
    if _trace:
        _ensure_ntff_hook()
    x = np.asarray(x, dtype=np.float32)
    qkv_w = np.asarray(qkv_w, dtype=np.float32)
    qkv_b = np.asarray(qkv_b, dtype=np.float32)
    out_w = np.asarray(out_w, dtype=np.float32)
    out_b = np.asarray(out_b, dtype=np.float32)

    has_qkv_bias = bool(np.any(qkv_b[:2 * D]))
    nc = _get_nc(has_qkv_bias)
    in_maps = make_in_maps(x, qkv_w, qkv_b, out_w)
    res = run_bass_kernel_spmd(nc, in_maps, core_ids=list(range(NCORES)),
                               trace=_trace)
    y = np.zeros((B, S, D), dtype=np.float32)
    for c in range(NCORES):
        y[c // (NCORES // B)] += np.asarray(res.results[c]["y"],
                                            dtype=np.float32)
    y += out_b
    if _trace:
        kernel.last_results = res
    return y


# revision 17
# speedup vs baseline: 1.0361x; 1.0361x over previous
"""Causal self-attention Trainium2 kernel (Bass/Tile), 8 NeuronCores.

Problem: B=2, S=2048, D=1024, H=16 heads (hd=64), fp32.
    qkv = x @ qkv_w + qkv_b ; per-head causal attention ; y = out @ out_proj + out_b

Sharding (hybrid data x tensor parallel):
    8 cores = 2 batch groups x 4 head groups. Core c handles batch c//4 and
    the 4 heads [4*(c%4) .. 4*(c%4)+3]. Each core computes its partial
    out-projection y_c [S, D] in bf16; host sums the 4 partials per batch
    (in fp32) + out_b.

v2 design (bf16 everywhere on the PE, balanced engine usage):
    - all matmuls in bf16 (full PE rate at any free size, half the DMA/SBUF)
    - phase 1 (projections) streams x per-ko with ko-OUTER accumulation over
      8 PSUM banks so the PE starts as soon as x[0] lands; PSUM->SBUF drains
      run on the otherwise-idle ACT engine
    - softmax: exp on ACT only (no Ln -> no activation-table thrash);
      denominators via the ones-column of V; 1/l via DVE
      reciprocal_approx_fast (~5x faster than nc.vector.reciprocal);
      broadcast of 1/l across partitions via a tiny K=2 f32r matmul
    - causal masking of diagonal 128-blocks via gpsimd.affine_select on the
      exp'd tile (keeps DVE free)
    - softmax-normalize + out-projection of q-tile jq are emitted as deferred
      units interleaved into the next tile's attention loop, so the PE never
      stalls on the normalization chain
"""

import os
import sys
from collections import deque

for _p in ("/opt/trn_rl_repo", "/root/.axon_site/_ro/trn_rl_repo"):
    if os.path.isdir(_p) and _p not in sys.path:
        sys.path.insert(0, _p)

import numpy as np
import ml_dtypes
from contextlib import ExitStack

import concourse.bass as bass
import concourse.tile as tile
from concourse import bacc, mybir
from concourse.bass_utils import run_bass_kernel_spmd

B, S, D = 2, 2048, 1024
H, HD = 16, 64
NCORES = 8
LOCAL_H = 4           # heads per core
P = 128
KO = D // P           # 8 contraction sub-tiles for the projections
NQ = S // 512         # 4 q-tiles of 512
NKT = S // P          # 16 k-blocks of 128
F32 = mybir.dt.float32
F32R = mybir.dt.float32r
BF16 = mybir.dt.bfloat16
AF = mybir.ActivationFunctionType
ALU = mybir.AluOpType
SCALE = 1.0 / np.sqrt(HD)


def _emit(tc, nc, xT, wqk, wv, wo, bqk, b65, onesd, sel2, y, has_qkv_bias):
    with ExitStack() as ctx:
        consts = ctx.enter_context(tc.tile_pool(name="consts", bufs=1))
        persis = ctx.enter_context(tc.tile_pool(name="persist", bufs=1))
        xstack = ctx.enter_context(ExitStack())
        xpool = xstack.enter_context(tc.tile_pool(name="xp", bufs=KO))
        ppstack = ctx.enter_context(ExitStack())
        pp = ppstack.enter_context(tc.tile_pool(name="pp", bufs=8, space="PSUM"))

        # ---- loads: wqk + early x tiles first (weights for later phases
        # are issued after the x stream so they don't steal DMA bandwidth)
        wqk_sb = consts.tile([P, KO, 512], BF16)
        nc.gpsimd.dma_start(wqk_sb[:], wqk.rearrange("(ko p) m -> p ko m", p=P))
        x_sb = []
        for ko in range(KO):
            t = xpool.tile([P, S], BF16, tag="x", name=f"x{ko}")
            x_sb.append(t)
        for ko in range(4):
            nc.sync.dma_start(x_sb[ko][:], xT[ko * P:(ko + 1) * P, :])
        b65_sb = consts.tile([1, 260], BF16)
        nc.scalar.dma_start(b65_sb[:], b65[None, :])
        ones_sb = consts.tile([1, P], BF16)
        nc.scalar.dma_start(ones_sb[:], onesd[None, :])
        onesr_sb = consts.tile([1, P], F32R)
        nc.scalar.dma_start(onesr_sb[:], sel2[0:1, :])
        for ko in range(4, KO):
            nc.sync.dma_start(x_sb[ko][:], xT[ko * P:(ko + 1) * P, :])
        wv_sb = consts.tile([P, KO, 260], BF16)
        nc.scalar.dma_start(wv_sb[:], wv.rearrange("(ko p) m -> p ko m", p=P))
        wo_sb = consts.tile([P, 2, D], BF16)
        nc.scalar.dma_start(wo_sb[:], wo.rearrange("(ks p) n -> p ks n", p=P))
        if has_qkv_bias:
            bqk_sb = consts.tile([P, 4], F32)
            nc.scalar.dma_start(bqk_sb[:], bqk.rearrange("(m p) -> p m", p=P))

        # persistent activations
        qkT = persis.tile([P, 4, S], BF16)       # m 0,1: qT(h0..h3); 2,3: kT
        v_all = persis.tile([P, NKT, LOCAL_H, 65], BF16)  # [k-part, kt, lh, hd|ones]
        outT = persis.tile([P, 2, S], BF16)      # attention out^T (out-proj lhsT)

        # ---- phase 1: projections, ko-outer over 8 PSUM banks ----
        # qkT[m] = (wqk[:, m-slice]).T @ xT, two passes of 8 (m, n) groups.
        # Each pass is ko-split (0-3, then 4-7) so the PE starts as soon as
        # the first x tiles land instead of waiting for the full x stream.
        for half in range(2):
            groups = [(m, 2 * half + nn) for m in range(4) for nn in range(2)]
            ts = [pp.tile([P, 512], F32, tag="p", name=f"qk{half}_{g}")
                  for g in range(8)]
            for ko in range(KO):
                for g, (m, n) in enumerate(groups):
                    nc.tensor.matmul(
                        ts[g][:],
                        wqk_sb[:, ko, m * P:(m + 1) * P],
                        x_sb[ko][:, n * 512:(n + 1) * 512],
                        start=(ko == 0), stop=(ko == KO - 1),
                    )
            for g, (m, n) in enumerate(groups):
                dst = qkT[:, m, n * 512:(n + 1) * 512]
                if has_qkv_bias:
                    nc.scalar.activation(dst, ts[g][:], AF.Identity,
                                         bias=bqk_sb[:, m:m + 1])
                else:
                    nc.scalar.copy(dst, ts[g][:])

        # v projection (natural layout, ones/bias row via K=1 matmul).
        # Second half drains on DVE so the ACT queue is clear for the first
        # attention exp.
        for half in range(2):
            mts = [8 * half + g for g in range(8)]
            ts = [pp.tile([P, 512], F32, tag="p", name=f"v{half}_{g}")
                  for g in range(8)]
            for ko in range(KO):
                for g, mt in enumerate(mts):
                    nc.tensor.matmul(
                        ts[g][:, 0:260],
                        x_sb[ko][:, mt * P:(mt + 1) * P],
                        wv_sb[:, ko, :],
                        start=(ko == 0), stop=False,
                    )
            for g, mt in enumerate(mts):
                nc.tensor.matmul(ts[g][:, 0:260], ones_sb[:1, :], b65_sb[:1, :],
                                 start=False, stop=True)
                (nc.scalar.copy if half == 0 else nc.vector.tensor_copy)(
                    v_all[:, mt, :, :],
                    ts[g][:, 0:260].rearrange("p (h d) -> p h d", h=LOCAL_H),
                )

        # x tiles + phase-1 psum are dead; release for the attention pools
        xstack.close()
        ppstack.close()

        psA = ctx.enter_context(tc.tile_pool(name="psA", bufs=2, space="PSUM"))
        psB = ctx.enter_context(tc.tile_pool(name="psB", bufs=3, space="PSUM"))
        psC = ctx.enter_context(tc.tile_pool(name="psC", bufs=1, space="PSUM"))
        work = ctx.enter_context(tc.tile_pool(name="work", bufs=4))
        small = ctx.enter_context(tc.tile_pool(name="small", bufs=2))
        ypool = ctx.enter_context(tc.tile_pool(name="yp", bufs=4))

        units = deque()

        def unit_rb_norm(jq, hp, st, rrr, i):
            def emit():
                rbp = psC.tile([P, 512], F32, tag="t", name="rbp")
                nc.tensor.matmul(rbp[0:64, :], onesr_sb[:1, 0:64],
                                 rrr[:1, :], start=True, stop=True)
                nc.vector.tensor_tensor(
                    outT[64 * i:64 * i + 64, hp, jq * 512:(jq + 1) * 512],
                    st[0:64, :], rbp[0:64, :], ALU.mult)
            return emit

        def unit_outproj(jq, mt, n2):
            def emit():
                pso = psA.tile([P, 2, 512], F32, tag="s", name="pso")[:, 0, :]
                for ks in range(2):
                    nc.tensor.matmul(
                        pso,
                        outT[:, ks, mt * P:(mt + 1) * P],
                        wo_sb[:, ks, n2 * 512:(n2 + 1) * 512],
                        start=(ks == 0), stop=(ks == 1),
                    )
                yt = ypool.tile([P, 512], BF16, tag="y", name="yt")
                nc.vector.tensor_copy(yt[:], pso)
                nc.sync.dma_start(
                    y[mt * P:(mt + 1) * P, n2 * 512:(n2 + 1) * 512], yt[:])
            return emit

        def emit_pv(po, jq, hp, kt, et, f0, last_kt):
            for i in range(2):
                nc.tensor.matmul(
                    po[i][:, f0:512],
                    v_all[:, kt, 2 * hp + i, :],
                    et[:, i, f0:512],
                    start=(kt == 0), stop=(kt == last_kt),
                )

        # ---- phase 2: attention with deferred normalize/out-proj units ----
        for jq in range(NQ):
            for hp in range(2):
                last_kt = 4 * jq + 3
                po = [psB.tile([65, 512], F32, tag="o", name=f"po{jq}{hp}{i_}")
                      for i_ in range(2)]
                pend = deque()
                for kt in range(last_kt + 1):
                    rel = kt - 4 * jq
                    f0 = 128 * rel if rel > 0 else 0
                    ps = psA.tile([P, 2, 512], F32, tag="s", name="ps")
                    for i in range(2):
                        poff = 64 * i
                        nc.tensor.matmul(
                            ps[:, i, f0:512],
                            qkT[poff:poff + 64, 2 + hp, kt * P:(kt + 1) * P],
                            qkT[poff:poff + 64, hp,
                                jq * 512 + f0:(jq + 1) * 512],
                            start=True, stop=True,
                        )
                    et = work.tile([P, 2, 512], BF16, tag="e", name="et")
                    nc.scalar.activation(et[:, :, f0:512], ps[:, :, f0:512],
                                         AF.Exp, scale=float(SCALE))
                    if rel >= 0:   # zero the 128-wide triangle at [f0, f0+128)
                        nc.gpsimd.affine_select(
                            out=et[:, :, f0:f0 + 128],
                            in_=et[:, :, f0:f0 + 128],
                            pattern=[[0, 2], [1, P]],
                            compare_op=ALU.is_ge, fill=0.0, base=0,
                            channel_multiplier=-1,
                        )
                    pend.append((kt, et, f0))
                    if len(pend) > 2:   # 2-deep score lookahead ahead of PV
                        emit_pv(po, jq, hp, *pend.popleft(), last_kt)
                    if units:   # fill PE with deferred work every iteration
                        units.popleft()()
                while pend:
                    emit_pv(po, jq, hp, *pend.popleft(), last_kt)
                # stage po out of PSUM on the ACT engine (frees the bank
                # fast), grab denominators, 1/l via fast DVE approx
                for i in range(2):
                    stt = work.tile([65, 512], F32, tag="st", name="st")
                    nc.vector.tensor_copy(stt[:], po[i][:])
                    lcp = small.tile([1, 512], F32, tag="lcp", name="lcp")
                    nc.vector.tensor_copy(lcp[:], po[i][64:65, :])
                    rr = small.tile([1, 512], F32, tag="rr", name="rr")
                    nc.vector.reciprocal_approx_fast(rr[:], lcp[:])
                    rrr = small.tile([1, 512], F32R, tag="rrr", name="rrr")
                    nc.vector.tensor_copy(rrr[:], rr[:])
                    units.append(unit_rb_norm(jq, hp, stt, rrr, i))
            for mt in range(4 * jq, 4 * jq + 4):
                for n2 in range(2):
                    units.append(unit_outproj(jq, mt, n2))
        while units:
            units.popleft()()


def build_nc(has_qkv_bias):
    nc = bacc.Bacc("TRN2", target_bir_lowering=False, debug=False,
                   num_devices=NCORES)
    xT = nc.dram_tensor("xT", [D, S], BF16, kind="ExternalInput")
    wqk = nc.dram_tensor("wqk", [D, 512], BF16, kind="ExternalInput")
    wv = nc.dram_tensor("wv", [D, 260], BF16, kind="ExternalInput")
    wo = nc.dram_tensor("wo", [2 * P, D], BF16, kind="ExternalInput")
    bqk = nc.dram_tensor("bqk", [512], F32, kind="ExternalInput")
    b65 = nc.dram_tensor("b65", [260], BF16, kind="ExternalInput")
    onesd = nc.dram_tensor("onesd", [P], BF16, kind="ExternalInput")
    sel2 = nc.dram_tensor("sel2", [2, P], F32R, kind="ExternalInput")
    y = nc.dram_tensor("y", [S, D], BF16, kind="ExternalOutput")
    with tile.TileContext(nc) as tc:
        _emit(tc, nc, xT.ap(), wqk.ap(), wv.ap(), wo.ap(), bqk.ap(), b65.ap(),
              onesd.ap(), sel2.ap(), y.ap(), has_qkv_bias)
    nc.compile()
    return nc


_NC_CACHE = {}


def _get_nc(has_qkv_bias):
    key = bool(has_qkv_bias)
    if key not in _NC_CACHE:
        _NC_CACHE[key] = build_nc(key)
    return _NC_CACHE[key]


def _bf16(a):
    return np.ascontiguousarray(a, dtype=np.float32).astype(ml_dtypes.bfloat16)


def make_in_maps(x, qkv_w, qkv_b, out_w):
    """Per-core host-side sharding. Core c: batch c//4, heads 4*(c%4)..+3."""
    in_maps = []
    xTs = [_bf16(x[b].T) for b in range(B)]
    sel2 = np.zeros((2, P), np.float32)
    sel2[0, 0:64] = 1.0
    sel2[1, 64:128] = 1.0
    for c in range(NCORES):
        b = c // (NCORES // B)
        g = c % (NCORES // B)
        h0 = LOCAL_H * g
        cols = slice(h0 * HD, (h0 + LOCAL_H) * HD)
        wq = qkv_w[:, cols]
        wk = qkv_w[:, D:][:, cols]
        wv_ = qkv_w[:, 2 * D:][:, cols]
        bq = qkv_b[cols]
        bk = qkv_b[D:][cols]
        bv = qkv_b[2 * D:][cols]
        wv_pad = np.zeros((D, LOCAL_H, 65), np.float32)
        wv_pad[:, :, :64] = wv_.reshape(D, LOCAL_H, HD)
        b65_arr = np.zeros((LOCAL_H, 65), np.float32)
        b65_arr[:, :64] = bv.reshape(LOCAL_H, HD)
        b65_arr[:, 64] = 1.0
        in_maps.append({
            "xT": xTs[b],
            "wqk": _bf16(np.concatenate([wq, wk], axis=1)),
            "wv": _bf16(wv_pad.reshape(D, LOCAL_H * 65)),
            "wo": _bf16(out_w[cols, :]),
            "bqk": np.ascontiguousarray(np.concatenate([bq, bk])),
            "b65": _bf16(b65_arr.reshape(-1)),
            "onesd": np.ones(P, ml_dtypes.bfloat16),
            "sel2": sel2,
        })
    return in_maps


def _ensure_ntff_hook():
    """Provide antenv.axon_hooks (missing in this image) so trace=True works."""
    try:
        from antenv.axon_hooks import get_axon_ntff_profile_hook  # noqa: F401
        return
    except ImportError:
        pass
    import types
    import antenv
    mod = types.ModuleType("antenv.axon_hooks")
    holder = {"hook": None}
    mod.set_axon_ntff_profile_hook = lambda h: holder.__setitem__("hook", h)
    mod.get_axon_ntff_profile_hook = lambda: holder["hook"]
    sys.modules["antenv.axon_hooks"] = mod
    antenv.axon_hooks = mod
    try:
        from trn_agent_boot.trn_boot import _ntff_profile_via_ctypes
        so = "/opt/axon/libaxon_pjrt.so"
        if os.path.exists(so):
            mod.set_axon_ntff_profile_hook(_ntff_profile_via_ctypes(so))
    except Exception:
        pass


def kernel(x, qkv_w, qkv_b, out_w, out_b, _trace=False):
    if _trace:
        _ensure_ntff_hook()
    x = np.asarray(x, dtype=np.float32)
    qkv_w = np.asarray(qkv_w, dtype=np.float32)
    qkv_b = np.asarray(qkv_b, dtype=np.float32)
    out_w = np.asarray(out_w, dtype=np.float32)
    out_b = np.asarray(out_b, dtype=np.float32)

    has_qkv_bias = bool(np.any(qkv_b[:2 * D]))
    nc = _get_nc(has_qkv_bias)
    in_maps = make_in_maps(x, qkv_w, qkv_b, out_w)
    res = run_bass_kernel_spmd(nc, in_maps, core_ids=list(range(NCORES)),
                               trace=_trace)
    y = np.zeros((B, S, D), dtype=np.float32)
    for c in range(NCORES):
        y[c // (NCORES // B)] += np.asarray(res.results[c]["y"],
                                            dtype=np.float32)
    y += out_b
    if _trace:
        kernel.last_results = res
    return y


# revision 21
# speedup vs baseline: 1.0405x; 1.0043x over previous
"""Causal self-attention Trainium2 kernel (Bass/Tile), 8 NeuronCores.

Problem: B=2, S=2048, D=1024, H=16 heads (hd=64), fp32.
    qkv = x @ qkv_w + qkv_b ; per-head causal attention ; y = out @ out_proj + out_b

Sharding (hybrid data x tensor parallel):
    8 cores = 2 batch groups x 4 head groups. Core c handles batch c//4 and
    the 4 heads [4*(c%4) .. 4*(c%4)+3]. Each core computes its partial
    out-projection y_c [S, D] in bf16; host sums the 4 partials per batch
    (in fp32) + out_b.

v2 design (bf16 everywhere on the PE, balanced engine usage):
    - all matmuls in bf16 (full PE rate at any free size, half the DMA/SBUF)
    - phase 1 (projections) streams x per-ko with ko-OUTER accumulation over
      8 PSUM banks so the PE starts as soon as x[0] lands; PSUM->SBUF drains
      run on the otherwise-idle ACT engine
    - softmax: exp on ACT only (no Ln -> no activation-table thrash);
      denominators via the ones-column of V; 1/l via DVE
      reciprocal_approx_fast (~5x faster than nc.vector.reciprocal);
      broadcast of 1/l across partitions via a tiny K=2 f32r matmul
    - causal masking of diagonal 128-blocks via gpsimd.affine_select on the
      exp'd tile (keeps DVE free)
    - softmax-normalize + out-projection of q-tile jq are emitted as deferred
      units interleaved into the next tile's attention loop, so the PE never
      stalls on the normalization chain
"""

import os
import sys
from collections import deque

for _p in ("/opt/trn_rl_repo", "/root/.axon_site/_ro/trn_rl_repo"):
    if os.path.isdir(_p) and _p not in sys.path:
        sys.path.insert(0, _p)

import numpy as np
import ml_dtypes
from contextlib import ExitStack

import concourse.bass as bass
import concourse.tile as tile
from concourse import bacc, mybir
from concourse.bass_utils import run_bass_kernel_spmd

B, S, D = 2, 2048, 1024
H, HD = 16, 64
NCORES = 8
LOCAL_H = 4           # heads per core
P = 128
KO = D // P           # 8 contraction sub-tiles for the projections
NQ = S // 512         # 4 q-tiles of 512
NKT = S // P          # 16 k-blocks of 128
F32 = mybir.dt.float32
F32R = mybir.dt.float32r
BF16 = mybir.dt.bfloat16
AF = mybir.ActivationFunctionType
ALU = mybir.AluOpType
SCALE = 1.0 / np.sqrt(HD)


def _emit(tc, nc, xT, wqk, wv, wo, bqk, b65, onesd, sel2, y, has_qkv_bias):
    with ExitStack() as ctx:
        consts = ctx.enter_context(tc.tile_pool(name="consts", bufs=1))
        persis = ctx.enter_context(tc.tile_pool(name="persist", bufs=1))
        xstack = ctx.enter_context(ExitStack())
        xpool = xstack.enter_context(tc.tile_pool(name="xp", bufs=KO))
        ppstack = ctx.enter_context(ExitStack())
        pp = ppstack.enter_context(tc.tile_pool(name="pp", bufs=8, space="PSUM"))

        # ---- loads: wqk first, then the x stream in half-tile chunks across
        # two issuing sequencers (per-dma_start queue parallelism is limited,
        # so chunking parallelizes the early tiles); weights for later phases
        # are issued after x so they don't steal DMA bandwidth.
        wqk_sb = consts.tile([P, KO, 512], BF16)
        nc.gpsimd.dma_start(wqk_sb[:, :, 0:256],
                            wqk.rearrange("(ko p) m -> p ko m", p=P)[:, :, 0:256])
        nc.gpsimd.dma_start(wqk_sb[:, :, 256:512],
                            wqk.rearrange("(ko p) m -> p ko m", p=P)[:, :, 256:512])
        x_sb = []
        for ko in range(KO):
            t = xpool.tile([P, S], BF16, tag="x", name=f"x{ko}")
            x_sb.append(t)
        for ko in range(KO):
            src = xT[ko * P:(ko + 1) * P, :]
            nc.sync.dma_start(x_sb[ko][:, 0:1024], src[:, 0:1024])
            nc.scalar.dma_start(x_sb[ko][:, 1024:2048], src[:, 1024:2048])
        b65_sb = consts.tile([1, 260], BF16)
        nc.scalar.dma_start(b65_sb[:], b65[None, :])
        ones_sb = consts.tile([1, P], BF16)
        nc.scalar.dma_start(ones_sb[:], onesd[None, :])
        onesr_sb = consts.tile([1, P], F32R)
        nc.scalar.dma_start(onesr_sb[:], sel2[0:1, :])
        wv_sb = consts.tile([P, KO, 260], BF16)
        nc.scalar.dma_start(wv_sb[:], wv.rearrange("(ko p) m -> p ko m", p=P))
        wo_sb = consts.tile([P, 2, D], BF16)
        nc.scalar.dma_start(wo_sb[:], wo.rearrange("(ks p) n -> p ks n", p=P))
        if has_qkv_bias:
            bqk_sb = consts.tile([P, 4], F32)
            nc.scalar.dma_start(bqk_sb[:], bqk.rearrange("(m p) -> p m", p=P))

        # persistent activations
        qkT = persis.tile([P, 4, S], BF16)       # m 0,1: qT(h0..h3); 2,3: kT
        v_all = persis.tile([P, NKT, LOCAL_H, 65], BF16)  # [k-part, kt, lh, hd|ones]
        outT = persis.tile([P, 2, S], BF16)      # attention out^T (out-proj lhsT)

        # ---- phase 1: projections, ko-outer over 8 PSUM banks ----
        # qkT[m] = (wqk[:, m-slice]).T @ xT, two passes of 8 (m, n) groups.
        # Each pass is ko-split (0-3, then 4-7) so the PE starts as soon as
        # the first x tiles land instead of waiting for the full x stream.
        for half in range(2):
            groups = [(m, 2 * half + nn) for m in range(4) for nn in range(2)]
            ts = [pp.tile([P, 512], F32, tag="p", name=f"qk{half}_{g}")
                  for g in range(8)]
            for ko in range(KO):
                for g, (m, n) in enumerate(groups):
                    nc.tensor.matmul(
                        ts[g][:],
                        wqk_sb[:, ko, m * P:(m + 1) * P],
                        x_sb[ko][:, n * 512:(n + 1) * 512],
                        start=(ko == 0), stop=(ko == KO - 1),
                    )
            for g, (m, n) in enumerate(groups):
                dst = qkT[:, m, n * 512:(n + 1) * 512]
                if has_qkv_bias:
                    nc.scalar.activation(dst, ts[g][:], AF.Identity,
                                         bias=bqk_sb[:, m:m + 1])
                else:
                    nc.scalar.copy(dst, ts[g][:])

        # v projection (natural layout, ones/bias row via K=1 matmul).
        # Second half drains on DVE so the ACT queue is clear for the first
        # attention exp.
        for half in range(2):
            mts = [8 * half + g for g in range(8)]
            ts = [pp.tile([P, 512], F32, tag="p", name=f"v{half}_{g}")
                  for g in range(8)]
            for ko in range(KO):
                for g, mt in enumerate(mts):
                    nc.tensor.matmul(
                        ts[g][:, 0:260],
                        x_sb[ko][:, mt * P:(mt + 1) * P],
                        wv_sb[:, ko, :],
                        start=(ko == 0), stop=False,
                    )
            for g, mt in enumerate(mts):
                nc.tensor.matmul(ts[g][:, 0:260], ones_sb[:1, :], b65_sb[:1, :],
                                 start=False, stop=True)
                (nc.scalar.copy if half == 0 else nc.vector.tensor_copy)(
                    v_all[:, mt, :, :],
                    ts[g][:, 0:260].rearrange("p (h d) -> p h d", h=LOCAL_H),
                )

        # x tiles + phase-1 psum are dead; release for the attention pools
        xstack.close()
        ppstack.close()

        psA = ctx.enter_context(tc.tile_pool(name="psA", bufs=2, space="PSUM"))
        psB = ctx.enter_context(tc.tile_pool(name="psB", bufs=3, space="PSUM"))
        psC = ctx.enter_context(tc.tile_pool(name="psC", bufs=1, space="PSUM"))
        work = ctx.enter_context(tc.tile_pool(name="work", bufs=5))
        small = ctx.enter_context(tc.tile_pool(name="small", bufs=2))
        ypool = ctx.enter_context(tc.tile_pool(name="yp", bufs=4))

        units = deque()

        def unit_rb_norm(jq, hp, st, rrr, i):
            def emit():
                rbp = psC.tile([P, 512], F32, tag="t", name="rbp")
                nc.tensor.matmul(rbp[0:64, :], onesr_sb[:1, 0:64],
                                 rrr[:1, :], start=True, stop=True)
                nc.vector.tensor_tensor(
                    outT[64 * i:64 * i + 64, hp, jq * 512:(jq + 1) * 512],
                    st[0:64, :], rbp[0:64, :], ALU.mult)
            return emit

        def unit_outproj(jq, mt, n2):
            def emit():
                pso = psA.tile([P, 2, 512], F32, tag="s", name="pso")[:, 0, :]
                for ks in range(2):
                    nc.tensor.matmul(
                        pso,
                        outT[:, ks, mt * P:(mt + 1) * P],
                        wo_sb[:, ks, n2 * 512:(n2 + 1) * 512],
                        start=(ks == 0), stop=(ks == 1),
                    )
                yt = ypool.tile([P, 512], BF16, tag="y", name="yt")
                nc.vector.tensor_copy(yt[:], pso)
                nc.sync.dma_start(
                    y[mt * P:(mt + 1) * P, n2 * 512:(n2 + 1) * 512], yt[:])
            return emit

        def emit_pv(po, jq, hp, kt, et, f0, last_kt):
            for i in range(2):
                nc.tensor.matmul(
                    po[i][:, f0:512],
                    v_all[:, kt, 2 * hp + i, :],
                    et[:, i, f0:512],
                    start=(kt == 0), stop=(kt == last_kt),
                )

        # ---- phase 2: attention with deferred normalize/out-proj units ----
        for jq in range(NQ):
            for hp in range(2):
                last_kt = 4 * jq + 3
                po = [psB.tile([65, 512], F32, tag="o", name=f"po{jq}{hp}{i_}")
                      for i_ in range(2)]
                pend = deque()
                for kt in range(last_kt + 1):
                    rel = kt - 4 * jq
                    f0 = 128 * rel if rel > 0 else 0
                    ps = psA.tile([P, 2, 512], F32, tag="s", name="ps")
                    for i in range(2):
                        poff = 64 * i
                        nc.tensor.matmul(
                            ps[:, i, f0:512],
                            qkT[poff:poff + 64, 2 + hp, kt * P:(kt + 1) * P],
                            qkT[poff:poff + 64, hp,
                                jq * 512 + f0:(jq + 1) * 512],
                            start=True, stop=True,
                        )
                    et = work.tile([P, 2, 512], BF16, tag="e", name="et")
                    nc.scalar.activation(et[:, :, f0:512], ps[:, :, f0:512],
                                         AF.Exp, scale=float(SCALE))
                    if rel >= 0:   # zero the 128-wide triangle at [f0, f0+128)
                        nc.gpsimd.affine_select(
                            out=et[:, :, f0:f0 + 128],
                            in_=et[:, :, f0:f0 + 128],
                            pattern=[[0, 2], [1, P]],
                            compare_op=ALU.is_ge, fill=0.0, base=0,
                            channel_multiplier=-1,
                        )
                    pend.append((kt, et, f0))
                    if len(pend) > 3:   # 3-deep score lookahead ahead of PV
                        emit_pv(po, jq, hp, *pend.popleft(), last_kt)
                    if units:   # fill PE with deferred work every iteration
                        units.popleft()()
                while pend:
                    emit_pv(po, jq, hp, *pend.popleft(), last_kt)
                # stage po out of PSUM on the ACT engine (frees the bank
                # fast), grab denominators, 1/l via fast DVE approx
                for i in range(2):
                    stt = work.tile([65, 512], F32, tag="st", name="st")
                    nc.vector.tensor_copy(stt[:], po[i][:])
                    lcp = small.tile([1, 512], F32, tag="lcp", name="lcp")
                    nc.vector.tensor_copy(lcp[:], po[i][64:65, :])
                    rr = small.tile([1, 512], F32, tag="rr", name="rr")
                    nc.vector.reciprocal_approx_fast(rr[:], lcp[:])
                    rrr = small.tile([1, 512], F32R, tag="rrr", name="rrr")
                    nc.vector.tensor_copy(rrr[:], rr[:])
                    units.append(unit_rb_norm(jq, hp, stt, rrr, i))
            for mt in range(4 * jq, 4 * jq + 4):
                for n2 in range(2):
                    units.append(unit_outproj(jq, mt, n2))
        while units:
            units.popleft()()


def build_nc(has_qkv_bias):
    nc = bacc.Bacc("TRN2", target_bir_lowering=False, debug=False,
                   num_devices=NCORES)
    xT = nc.dram_tensor("xT", [D, S], BF16, kind="ExternalInput")
    wqk = nc.dram_tensor("wqk", [D, 512], BF16, kind="ExternalInput")
    wv = nc.dram_tensor("wv", [D, 260], BF16, kind="ExternalInput")
    wo = nc.dram_tensor("wo", [2 * P, D], BF16, kind="ExternalInput")
    bqk = nc.dram_tensor("bqk", [512], F32, kind="ExternalInput")
    b65 = nc.dram_tensor("b65", [260], BF16, kind="ExternalInput")
    onesd = nc.dram_tensor("onesd", [P], BF16, kind="ExternalInput")
    sel2 = nc.dram_tensor("sel2", [2, P], F32R, kind="ExternalInput")
    y = nc.dram_tensor("y", [S, D], BF16, kind="ExternalOutput")
    with tile.TileContext(nc) as tc:
        _emit(tc, nc, xT.ap(), wqk.ap(), wv.ap(), wo.ap(), bqk.ap(), b65.ap(),
              onesd.ap(), sel2.ap(), y.ap(), has_qkv_bias)
    nc.compile()
    return nc


_NC_CACHE = {}


def _get_nc(has_qkv_bias):
    key = bool(has_qkv_bias)
    if key not in _NC_CACHE:
        _NC_CACHE[key] = build_nc(key)
    return _NC_CACHE[key]


def _bf16(a):
    return np.ascontiguousarray(a, dtype=np.float32).astype(ml_dtypes.bfloat16)


def make_in_maps(x, qkv_w, qkv_b, out_w):
    """Per-core host-side sharding. Core c: batch c//4, heads 4*(c%4)..+3."""
    in_maps = []
    xTs = [_bf16(x[b].T) for b in range(B)]
    sel2 = np.zeros((2, P), np.float32)
    sel2[0, 0:64] = 1.0
    sel2[1, 64:128] = 1.0
    for c in range(NCORES):
        b = c // (NCORES // B)
        g = c % (NCORES // B)
        h0 = LOCAL_H * g
        cols = slice(h0 * HD, (h0 + LOCAL_H) * HD)
        wq = qkv_w[:, cols]
        wk = qkv_w[:, D:][:, cols]
        wv_ = qkv_w[:, 2 * D:][:, cols]
        bq = qkv_b[cols]
        bk = qkv_b[D:][cols]
        bv = qkv_b[2 * D:][cols]
        wv_pad = np.zeros((D, LOCAL_H, 65), np.float32)
        wv_pad[:, :, :64] = wv_.reshape(D, LOCAL_H, HD)
        b65_arr = np.zeros((LOCAL_H, 65), np.float32)
        b65_arr[:, :64] = bv.reshape(LOCAL_H, HD)
        b65_arr[:, 64] = 1.0
        in_maps.append({
            "xT": xTs[b],
            "wqk": _bf16(np.concatenate([wq, wk], axis=1)),
            "wv": _bf16(wv_pad.reshape(D, LOCAL_H * 65)),
            "wo": _bf16(out_w[cols, :]),
            "bqk": np.ascontiguousarray(np.concatenate([bq, bk])),
            "b65": _bf16(b65_arr.reshape(-1)),
            "onesd": np.ones(P, ml_dtypes.bfloat16),
            "sel2": sel2,
        })
    return in_maps


def _ensure_ntff_hook():
    """Provide antenv.axon_hooks (missing in this image) so trace=True works."""
    try:
        from antenv.axon_hooks import get_axon_ntff_profile_hook  # noqa: F401
        return
    except ImportError:
        pass
    import types
    import antenv
    mod = types.ModuleType("antenv.axon_hooks")
    holder = {"hook": None}
    mod.set_axon_ntff_profile_hook = lambda h: holder.__setitem__("hook", h)
    mod.get_axon_ntff_profile_hook = lambda: holder["hook"]
    sys.modules["antenv.axon_hooks"] = mod
    antenv.axon_hooks = mod
    try:
        from trn_agent_boot.trn_boot import _ntff_profile_via_ctypes
        so = "/opt/axon/libaxon_pjrt.so"
        if os.path.exists(so):
            mod.set_axon_ntff_profile_hook(_ntff_profile_via_ctypes(so))
    except Exception:
        pass


def kernel(x, qkv_w, qkv_b, out_w, out_b, _trace=False):
    if _trace:
        _ensure_ntff_hook()
    x = np.asarray(x, dtype=np.float32)
    qkv_w = np.asarray(qkv_w, dtype=np.float32)
    qkv_b = np.asarray(qkv_b, dtype=np.float32)
    out_w = np.asarray(out_w, dtype=np.float32)
    out_b = np.asarray(out_b, dtype=np.float32)

    has_qkv_bias = bool(np.any(qkv_b[:2 * D]))
    nc = _get_nc(has_qkv_bias)
    in_maps = make_in_maps(x, qkv_w, qkv_b, out_w)
    res = run_bass_kernel_spmd(nc, in_maps, core_ids=list(range(NCORES)),
                               trace=_trace)
    y = np.zeros((B, S, D), dtype=np.float32)
    for c in range(NCORES):
        y[c // (NCORES // B)] += np.asarray(res.results[c]["y"],
                                            dtype=np.float32)
    y += out_b
    if _trace:
        kernel.last_results = res
    return y
